# revision 4
# baseline (speedup 1.0000x reference)
"""CapsuleLayer kernel for Trainium2 (8 NeuronCores, Bass/Tile).

Math: reference einsum("bhwf,fcd->bhwd", x, Wc) sums over BOTH f and c,
so it collapses to a single matmul:
    W_eff[f, d] = sum_c capsules.reshape(F, C, D)[f, c, d]
    out = x.reshape(-1, F) @ W_eff            # (100352, 256) @ (256, 16)

Distribution: data-parallel over flattened positions (batch*H*W), 12544
positions per core; the small capsule weight is replicated. Each core
receives its x shard pre-transposed to (F, PPC) so the contraction dim f
sits on SBUF partitions (the tensor engine contracts over partitions);
the core emits outT (16, PPC) which the host transposes back (6.4 MB).
"""

import numpy as np

import concourse.bass as bass  # noqa: F401  (engine types referenced via nc)
import concourse.tile as tile
from concourse import bacc, mybir
from concourse.bass_utils import run_bass_kernel_spmd

N_CORES = 8
B, H, W, F = 32, 56, 56, 256
NUM_CAPS, CAP_DIM = 10, 16
POS = B * H * W            # 100352
PPC = POS // N_CORES       # 12544 positions per core
NT = 1792                  # positions per outer chunk (7168 B/partition DMA)
NSUB = 4
SUB = NT // NSUB           # 448 = matmul moving free dim (<=512 fp32)
NOUT = PPC // NT           # 7 outer chunks
KC = F // 128              # 2 contraction chunks of 128

# float32r streams 1 row/cycle through the PE (vs 4 for exact fp32) at the
# cost of reduced multiply precision; flip here if accuracy requires.
USE_F32R = True

_cache = {}


def _build(use_f32r: bool):
    nc = bacc.Bacc(
        None,
        target_bir_lowering=False,
        debug=False,
        enable_asserts=False,
        num_devices=N_CORES,
    )
    mm_dt = mybir.dt.float32r if use_f32r else mybir.dt.float32

    xT = nc.dram_tensor("xT", [F, PPC], mybir.dt.float32, kind="ExternalInput")
    caps = nc.dram_tensor(
        "caps", [F, NUM_CAPS * CAP_DIM], mybir.dt.float32, kind="ExternalInput"
    )
    outT = nc.dram_tensor("outT", [CAP_DIM, PPC], mybir.dt.float32, kind="ExternalOutput")

    with tile.TileContext(nc) as tc:
        with (
            tc.tile_pool(name="const", bufs=1) as cpool,
            tc.tile_pool(name="xin", bufs=3) as xpool,
            tc.tile_pool(name="outp", bufs=3) as opool,
            tc.tile_pool(name="psum", bufs=4, space="PSUM") as pspool,
        ):
            # ---- W_eff = sum over capsules of the (F, C*D) weight --------
            ct = cpool.tile([128, KC, NUM_CAPS * CAP_DIM], mybir.dt.float32, tag="caps")
            nc.sync.dma_start(ct[:], caps.rearrange("(k p) c -> p k c", p=128))
            weff = cpool.tile([128, KC, CAP_DIM], mm_dt, tag="weff")
            with nc.allow_low_precision(reason="float32r is 4-byte; same width"):
                for k in range(KC):
                    # view (128, C*D) as (128, D, C) and reduce the capsule axis
                    nc.vector.reduce_sum(
                        weff[:, k, :],
                        ct[:, k, :].rearrange("p (c d) -> p d c", c=NUM_CAPS),
                        axis=mybir.AxisListType.X,
                    )

            # ---- streaming matmul over position chunks -------------------
            for j in range(NOUT):
                cols = slice(j * NT, (j + 1) * NT)
                xt0 = xpool.tile([128, NT], mm_dt, tag="xt0")
                xt1 = xpool.tile([128, NT], mm_dt, tag="xt1")
                nc.sync.dma_start(xt0[:], xT[0:128, cols].bitcast(mm_dt))
                nc.sync.dma_start(xt1[:], xT[128:256, cols].bitcast(mm_dt))
                xts = (xt0, xt1)

                ob = opool.tile([CAP_DIM, NT], mybir.dt.float32, tag="ob")
                for s in range(NSUB):
                    sl = slice(s * SUB, (s + 1) * SUB)
                    ps = pspool.tile([CAP_DIM, SUB], mybir.dt.float32, tag="ps")
                    for k in range(KC):
                        nc.tensor.matmul(
                            ps[:],
                            weff[:, k, :],
                            xts[k][:, sl],
                            start=(k == 0),
                            stop=(k == KC - 1),
                        )
                    nc.vector.tensor_copy(ob[:, sl], ps[:])
                nc.sync.dma_start(outT[:, cols], ob[:])

    nc.compile()
    return nc


def _get_nc(use_f32r: bool):
    if use_f32r not in _cache:
        _cache[use_f32r] = _build(use_f32r)
    return _cache[use_f32r]


def run(x, capsules, trace=False, trace_cores=None, use_f32r=None):
    """Shard, execute on 8 cores, gather. Returns (out, BassKernelResults)."""
    if use_f32r is None:
        use_f32r = USE_F32R
    nc = _get_nc(use_f32r)

    x = np.asarray(x, dtype=np.float32)
    capsules = np.asarray(capsules, dtype=np.float32)
    xf = x.reshape(POS, F)
    caps2 = np.ascontiguousarray(capsules.reshape(F, NUM_CAPS * CAP_DIM))
    xT_full = xf.T  # view; per-core slices are copied once during input concat

    in_maps = [
        {"xT": xT_full[:, c * PPC : (c + 1) * PPC], "caps": caps2}
        for c in range(N_CORES)
    ]
    res = run_bass_kernel_spmd(
        nc,
        in_maps,
        core_ids=list(range(N_CORES)),
        trace=trace,
        trace_cores=trace_cores,
    )
    out = np.empty((POS, CAP_DIM), dtype=np.float32)
    for c in range(N_CORES):
        out[c * PPC : (c + 1) * PPC] = res.results[c]["outT"].T
    return out.reshape(B, H, W, CAP_DIM), res


def kernel(x, capsules):
    out, _ = run(x, capsules)
    return out


# revision 6
# speedup vs baseline: 1.1284x; 1.1284x over previous
"""CapsuleLayer kernel for Trainium2 (8 NeuronCores, Bass/Tile).

Math: reference einsum("bhwf,fcd->bhwd", x, Wc) sums over BOTH f and c,
so it collapses to a single matmul:
    W_eff[f, d] = sum_c capsules.reshape(F, C, D)[f, c, d]
    out = x.reshape(-1, F) @ W_eff            # (100352, 256) @ (256, 16)

Distribution: data-parallel over flattened positions (batch*H*W), 12544
positions per core; the small capsule weight is replicated. Each core
receives its x shard pre-transposed to (F, PPC) so the contraction dim f
sits on SBUF partitions (the tensor engine contracts over partitions);
the core emits outT (16, PPC) which the host transposes back (6.4 MB).
"""

import numpy as np

import concourse.bass as bass  # noqa: F401  (engine types referenced via nc)
import concourse.tile as tile
from concourse import bacc, mybir
from concourse.bass_utils import run_bass_kernel_spmd

N_CORES = 8
B, H, W, F = 32, 56, 56, 256
NUM_CAPS, CAP_DIM = 10, 16
POS = B * H * W            # 100352
PPC = POS // N_CORES       # 12544 positions per core
NT = 1792                  # positions per outer chunk (7168 B/partition DMA)
NSUB = 4
SUB = NT // NSUB           # 448 = matmul moving free dim (<=512 fp32)
NOUT = PPC // NT           # 7 outer chunks
KC = F // 128              # 2 contraction chunks of 128

# float32r streams 1 row/cycle through the PE (vs 4 for exact fp32) at the
# cost of reduced multiply precision; flip here if accuracy requires.
USE_F32R = True

_cache = {}


def _build(use_f32r: bool):
    nc = bacc.Bacc(
        None,
        target_bir_lowering=False,
        debug=False,
        enable_asserts=False,
        num_devices=N_CORES,
    )
    mm_dt = mybir.dt.float32r if use_f32r else mybir.dt.float32

    xT = nc.dram_tensor("xT", [F, PPC], mybir.dt.float32, kind="ExternalInput")
    caps = nc.dram_tensor(
        "caps", [F, NUM_CAPS * CAP_DIM], mybir.dt.float32, kind="ExternalInput"
    )
    outT = nc.dram_tensor("outT", [CAP_DIM, PPC], mybir.dt.float32, kind="ExternalOutput")

    with tile.TileContext(nc) as tc:
        with (
            tc.tile_pool(name="const", bufs=1) as cpool,
            tc.tile_pool(name="xin", bufs=NOUT) as xpool,
            tc.tile_pool(name="outp", bufs=3) as opool,
            tc.tile_pool(name="psum", bufs=2, space="PSUM") as pspool,
        ):
            # ---- W_eff = sum over capsules of the (F, C*D) weight --------
            ct = cpool.tile([128, KC, NUM_CAPS * CAP_DIM], mybir.dt.float32, tag="caps")
            nc.sync.dma_start(ct[:], caps.rearrange("(k p) c -> p k c", p=128))
            weff = cpool.tile([128, KC, CAP_DIM], mm_dt, tag="weff")
            with nc.allow_low_precision(reason="float32r is 4-byte; same width"):
                for k in range(KC):
                    # view (128, C*D) as (128, D, C) and reduce the capsule axis
                    nc.vector.reduce_sum(
                        weff[:, k, :],
                        ct[:, k, :].rearrange("p (c d) -> p d c", c=NUM_CAPS),
                        axis=mybir.AxisListType.X,
                    )

            # ---- streaming matmul over position chunks -------------------
            # all chunk tiles resident (12.85 MB < 24 MB SBUF): the input
            # DMAs have no buffer-recycle deps, so they queue back-to-back
            # on the sync HWDGE ring and stream at line rate.
            xT_v = xT.rearrange("(k p) n -> p k n", k=KC)  # [128, KC, PPC]
            xts = []
            for j in range(NOUT):
                cols = slice(j * NT, (j + 1) * NT)
                xt = xpool.tile([128, KC, NT], mm_dt, tag="xt")
                nc.sync.dma_start(xt[:], xT_v[:, :, cols].bitcast(mm_dt))
                xts.append(xt)

            for j in range(NOUT):
                cols = slice(j * NT, (j + 1) * NT)
                xt = xts[j]
                # one PSUM tile = NSUB bank-aligned accumulation groups
                ps = pspool.tile([CAP_DIM, NSUB, 512], mybir.dt.float32, tag="ps")
                for s in range(NSUB):
                    sl = slice(s * SUB, (s + 1) * SUB)
                    for k in range(KC):
                        nc.tensor.matmul(
                            ps[:, s, 0:SUB],
                            weff[:, k, :],
                            xt[:, k, sl],
                            start=(k == 0),
                            stop=(k == KC - 1),
                        )
                ob = opool.tile([CAP_DIM, NSUB, SUB], mybir.dt.float32, tag="ob")
                nc.vector.tensor_copy(ob[:], ps[:, :, 0:SUB])
                # store on the scalar HWDGE ring to stay off the input ring
                nc.scalar.dma_start(
                    outT.rearrange("d (c s n) -> d c s n", c=NOUT, s=NSUB)[:, j],
                    ob[:],
                )

    nc.compile()
    return nc


def _get_nc(use_f32r: bool):
    if use_f32r not in _cache:
        _cache[use_f32r] = _build(use_f32r)
    return _cache[use_f32r]


def run(x, capsules, trace=False, trace_cores=None, use_f32r=None):
    """Shard, execute on 8 cores, gather. Returns (out, BassKernelResults)."""
    if use_f32r is None:
        use_f32r = USE_F32R
    nc = _get_nc(use_f32r)

    x = np.asarray(x, dtype=np.float32)
    capsules = np.asarray(capsules, dtype=np.float32)
    xf = x.reshape(POS, F)
    caps2 = np.ascontiguousarray(capsules.reshape(F, NUM_CAPS * CAP_DIM))
    xT_full = xf.T  # view; per-core slices are copied once during input concat

    in_maps = [
        {"xT": xT_full[:, c * PPC : (c + 1) * PPC], "caps": caps2}
        for c in range(N_CORES)
    ]
    res = run_bass_kernel_spmd(
        nc,
        in_maps,
        core_ids=list(range(N_CORES)),
        trace=trace,
        trace_cores=trace_cores,
    )
    out = np.empty((POS, CAP_DIM), dtype=np.float32)
    for c in range(N_CORES):
        out[c * PPC : (c + 1) * PPC] = res.results[c]["outT"].T
    return out.reshape(B, H, W, CAP_DIM), res


def kernel(x, capsules):
    out, _ = run(x, capsules)
    return out
